# revision 11
# baseline (speedup 1.0000x reference)
"""Trainium2 Bass kernel for batched tanh-query attention.

Per-batch computation (B=8, one batch per NeuronCore, pure data parallel):
    q = tanh(out_state)            [Q, H]    Q=K=2048, H=128
    S = q @ history.T              [Q, K]
    P = softmax(S, axis=K)
    attn = P @ history             [Q, H]

Flash-style in the transposed orientation S_T[k, q] so MM2 needs no
transpose of P.  The kernel is ACT(exp)-rate-limited; everything else is
scheduled around a steady one-exp-per-pair cadence:

  slot (q,p):  MM1 pair p (2x512-col), exp[128,1024] f32->bf16 (ACT),
               DVE lvl1 pair-add + GpSimd lvl2 quad-add for the softmax
               denominator, MM2 accumulation for pair p-2 (lag 2), d
               quad-matmuls at p=4..6 (+1 in next quarter), previous
               quarter's epilogue spread over p1..p3.

Input DMAs ride two rings in parallel (hn chunks on the Sync ring, os
chunks on the GpSimd ring) so the hn stream feeds ht transposes / MM2 at
pair rate while os lands for tanh.  PSUM: 2x2 (st) + 1 (acc) + 1 (dq)
+ 2 (transpose bounce) = 8 banks.
"""

import os
import sys

os.environ.setdefault("NEURON_RT_RESET_CORES", "1")
for _p in ("/opt/trn_rl_repo", "/opt/trn_rl_repo/concourse"):
    if _p not in sys.path:
        sys.path.insert(0, _p)

import numpy as np

N_CORES = 8
SEQ = 2048
H = 128
P = 128
T = SEQ // P          # 16 seq tiles
NQ = 4                # query quarters
QW = SEQ // NQ        # 512
QTPQ = QW // P        # 4 q-tiles per quarter
NPAIR = T // 2        # 8 kb-pairs per quarter

_CACHE = {}


def _build():
    from concourse import bacc, bass, masks, mybir, tile

    f32 = mybir.dt.float32
    bf16 = mybir.dt.bfloat16
    AF = mybir.ActivationFunctionType

    nc = bacc.Bacc("TRN2", target_bir_lowering=False, debug=False,
                   num_devices=N_CORES)
    os_d = nc.dram_tensor("out_state", (SEQ, H), f32, kind="ExternalInput")
    h_d = nc.dram_tensor("history", (SEQ, H), f32, kind="ExternalInput")
    a_d = nc.dram_tensor("attn", (SEQ, H), f32, kind="ExternalOutput")

    with tile.TileContext(nc) as tc:
        with (
            tc.tile_pool(name="const", bufs=1) as constp,
            tc.tile_pool(name="big", bufs=1) as bigp,
            tc.tile_pool(name="stage", bufs=1) as stagep,
            tc.tile_pool(name="work", bufs=2) as workp,
            tc.tile_pool(name="expool", bufs=4) as expool,
            tc.tile_pool(name="dtree", bufs=4) as dtreep,
            tc.tile_pool(name="stp", bufs=2, space=bass.MemorySpace.PSUM) as stp,
            tc.tile_pool(name="pacc", bufs=1, space=bass.MemorySpace.PSUM) as pacc,
            tc.tile_pool(name="psd", bufs=1, space=bass.MemorySpace.PSUM) as psd,
        ):
            # ---- input DMAs: two rings in parallel, ordered by need ----
            os_f = stagep.tile([P, T, H], f32, tag="osf")
            hn_f = stagep.tile([P, T, H], f32, tag="hnf")
            os_v = os_d[:].rearrange("(t p) h -> p t h", p=P)
            hn_v = h_d[:].rearrange("(t p) h -> p t h", p=P)
            # single fast ring (Sync), interleaved by need-time
            for (buf, view, a, b) in (
                (os_f, os_v, 0, 4),
                (hn_f, hn_v, 0, 4),
                (hn_f, hn_v, 4, 8),
                (os_f, os_v, 4, 8),
                (hn_f, hn_v, 8, 12),
                (hn_f, hn_v, 12, 16),
                (os_f, os_v, 8, 12),
                (os_f, os_v, 12, 16),
            ):
                nc.sync.dma_start(buf[:, a:b, :], view[:, a:b, :])

            # ---- constants ----
            id_bf = constp.tile([P, P], bf16, tag="idb")
            masks.make_identity(nc, id_bf[:])
            id_f32 = constp.tile([P, P], f32, tag="idf")
            masks.make_identity(nc, id_f32[:])
            ones_bf = constp.tile([P, P], bf16, tag="ones")
            nc.vector.memset(ones_bf[:], 1.0)

            # persistent bf16 operands
            hn = bigp.tile([P, T, P], bf16, tag="hn")    # [k_in, t, h] natural
            ht = bigp.tile([P, T, P], bf16, tag="ht")    # [h, t, k_in]
            qT = bigp.tile([P, T, P], bf16, tag="qT")    # [h, t, q_in]
            q_nat = bigp.tile([P, T, H], bf16, tag="qnat")

            # prologue preprocessing for the early tiles
            nc.scalar.activation(q_nat[:, 0:4, :], os_f[:, 0:4, :], AF.Tanh)

            # PE-transpose one [128,128] bf16 tile into transposed layout
            def ptranspose(dst, src):
                tp = psd.tile([P, P], bf16, tag="tp", name="tp", bufs=2)
                nc.tensor.transpose(tp[:], src, id_bf[:])
                nc.vector.tensor_copy(dst, tp[:])

            # upfront: tiles the first A-phase pair needs.  Order matters:
            # these DVE copies must come before the (DMA-gated) late casts
            # so the tp slots recycle promptly.
            for t in range(QTPQ):
                ptranspose(qT[:, t, :], q_nat[:, t, :])
            nc.vector.tensor_copy(hn[:, 0:4, :], hn_f[:, 0:4, :])
            ptranspose(ht[:, 0, :], hn[:, 0, :])
            ptranspose(ht[:, 1, :], hn[:, 1, :])

            # late hn casts, emitted in arrival order
            nc.vector.tensor_copy(hn[:, 4:8, :], hn_f[:, 4:8, :])
            nc.vector.tensor_copy(hn[:, 8:12, :], hn_f[:, 8:12, :])
            nc.gpsimd.tensor_copy(hn[:, 12:14, :], hn_f[:, 12:14, :])
            nc.gpsimd.tensor_copy(hn[:, 14:16, :], hn_f[:, 14:16, :])

            # aux transpose queue
            aux = []

            def tp_job(kind, t):
                def job():
                    src = hn if kind == "h" else q_nat
                    dst = ht if kind == "h" else qT
                    ptranspose(dst[:, t, :], src[:, t, :])
                return job

            aux.extend(tp_job("h", t) for t in range(2, T))
            aux.extend(tp_job("q", t) for t in range(QTPQ, T))

            def drain_aux(n):
                for _ in range(n):
                    if aux:
                        aux.pop(0)()

            # ---- per-quarter state ----
            ex_tiles = [[None] * NPAIR for _ in range(NQ)]
            lvl2s = [[None] * 4 for _ in range(NQ)]
            l1prev = [None] * NQ
            accs = [None] * NQ
            dqs = [None] * NQ
            aTs_t = [None] * NQ
            dsb_t = [None] * NQ

            def emit_pair(q, p):
                kb0 = 2 * p
                st = stp.tile([P, 2 * QW], f32, tag="st", name="st")
                rhs = qT[:, QTPQ * q: QTPQ * (q + 1), :]
                nc.tensor.matmul(st[:, 0:QW], ht[:, kb0, :], rhs,
                                 start=True, stop=True)
                nc.tensor.matmul(st[:, QW:], ht[:, kb0 + 1, :], rhs,
                                 start=True, stop=True)
                ex = expool.tile([P, 2 * QW], bf16, tag="ex", name="ex")
                nc.scalar.activation(ex[:], st[:], AF.Exp)
                ex_tiles[q][p] = ex
                # d partial sums: DVE pair add, GpSimd quad add
                t1 = dtreep.tile([P, QW], bf16, tag="l1", name="t1", bufs=2)
                nc.vector.tensor_add(t1[:], ex[:, 0:QW], ex[:, QW:])
                if l1prev[q] is None:
                    l1prev[q] = t1
                else:
                    t2 = dtreep.tile([P, QW], bf16, tag="l2", name="t2",
                                     bufs=4)
                    eng = nc.vector if q == NQ - 1 else nc.gpsimd
                    eng.tensor_add(t2[:], l1prev[q][:], t1[:])
                    l1prev[q] = None
                    lvl2s[q][p // 2] = t2

            def emit_mm2(q, kb):
                if accs[q] is None:
                    accs[q] = pacc.tile([P, QW], f32, tag="acc",
                                        name=f"acc{q}")
                ex = ex_tiles[q][kb // 2]
                nc.tensor.matmul(
                    accs[q][:], hn[:, kb, :],
                    ex[:, QW * (kb % 2): QW * (kb % 2 + 1)],
                    start=(kb == 0), stop=(kb == T - 1))

            def emit_dmm(q, j):
                if dqs[q] is None:
                    dqs[q] = psd.tile([P, QW], f32, tag="dq", name=f"dq{q}")
                nc.tensor.matmul(dqs[q][:], ones_bf[:], lvl2s[q][j][:],
                                 start=(j == 0), stop=(j == 3))

            def emit_epi_head(q):
                # move acc + d row out of PSUM, freeing acc/dq slots
                d_sb = workp.tile([1, QW], f32, tag="dsb", name=f"dsb{q}")
                nc.vector.tensor_copy(d_sb[:], dqs[q][0:1, :])
                dsb_t[q] = d_sb
                aTs = workp.tile([P, QW], bf16, tag="aTs", name=f"aTs{q}")
                nc.vector.tensor_copy(aTs[:], accs[q][:])
                aTs_t[q] = aTs

            def emit_epi_tile(q, t):
                dps = psd.tile([P, 1], f32, tag="tp", name="dps", bufs=2)
                nc.tensor.transpose(dps[:], dsb_t[q][0:1, P * t: P * (t + 1)],
                                    id_f32[0:1, 0:1])
                rc = workp.tile([P, 1], f32, tag="rc", name="rc", bufs=4)
                nc.vector.reciprocal(rc[:], dps[:])
                tp = psd.tile([P, P], bf16, tag="tp", name="etp", bufs=2)
                nc.tensor.transpose(tp[:], aTs_t[q][:, P * t: P * (t + 1)],
                                    id_bf[:])
                ot = workp.tile([P, P], f32, tag="ot", name="ot", bufs=4)
                nc.vector.tensor_scalar_mul(ot[:], tp[:], rc[:])
                row0 = q * QW + P * t
                nc.sync.dma_start(a_d[row0: row0 + P, :], ot[:])

            # ---- emission schedule ----
            for q in range(NQ):
                for p in range(NPAIR):
                    if q == 0 and p >= 1:
                        drain_aux(2)          # ht transposes, 1 pair ahead
                    emit_pair(q, p)
                    if p >= 2:
                        emit_mm2(q, 2 * p - 4)
                        emit_mm2(q, 2 * p - 3)
                    if p == 0 and q >= 1:
                        emit_mm2(q - 1, 12)
                        emit_mm2(q - 1, 13)
                        emit_dmm(q - 1, 0)
                        emit_dmm(q - 1, 1)
                    if p == 1 and q >= 1:
                        emit_mm2(q - 1, 14)
                        emit_mm2(q - 1, 15)
                        emit_dmm(q - 1, 2)
                        emit_dmm(q - 1, 3)
                        emit_epi_head(q - 1)
                    if q >= 1 and p in (2, 3):
                        emit_epi_tile(q - 1, 2 * (p - 2))
                        emit_epi_tile(q - 1, 2 * (p - 2) + 1)
                    if q == NQ - 1 and p in (5, 6, 7):
                        emit_dmm(q, p - 5)
                    # late tanh chunks, gated on the exp stream via a
                    # zero bias (0 * ex) so the scheduler cannot hoist them
                    # into a DMA wait ahead of the exps
                    gate_tile = {(0, 1): 4, (0, 6): 8, (1, 1): 12}.get((q, p))
                    if gate_tile is not None:
                        g = workp.tile([P, 1], f32, tag="gate", name="gate",
                                       bufs=2)
                        nc.vector.tensor_scalar_mul(
                            g[:], ex_tiles[q][p][:, 0:1], 0.0)
                        a = gate_tile
                        nc.scalar.activation(q_nat[:, a:a + 4, :],
                                             os_f[:, a:a + 4, :], AF.Tanh,
                                             bias=g[:])
                    if q == 0 and p == 7:
                        drain_aux(4)          # qT tiles 4-7 before A(1)
                    if q >= 1 and p in (3, 4, 5, 6):
                        drain_aux(1)          # remaining qT transposes

            # ---- tail: finish quarter 3 ----
            for kb in (12, 13, 14, 15):
                emit_mm2(3, kb)
            emit_dmm(3, 3)
            emit_epi_head(3)
            for t in range(QTPQ):
                emit_epi_tile(3, t)
            while aux:
                aux.pop(0)()

    nc.compile()
    return nc


def _get_nc():
    if "nc" not in _CACHE:
        _CACHE["nc"] = _build()
    return _CACHE["nc"]


def _run(out_state, history, trace=False):
    from concourse.bass_utils import run_bass_kernel_spmd

    nc = _get_nc()
    out_state = np.ascontiguousarray(out_state, dtype=np.float32)
    history = np.ascontiguousarray(history, dtype=np.float32)
    in_maps = [
        {"out_state": out_state[b], "history": history[b]}
        for b in range(N_CORES)
    ]
    res = run_bass_kernel_spmd(nc, in_maps, core_ids=list(range(N_CORES)),
                               trace=trace)
    attn = np.stack([res.results[b]["attn"] for b in range(N_CORES)], axis=0)
    return attn.astype(np.float32), res


def kernel(out_state, history):
    try:
        attn, _ = _run(out_state, history)
    except Exception:
        # one retry, e.g. if a previous process left a core wedged
        attn, _ = _run(out_state, history)
    return attn


# revision 12
# speedup vs baseline: 1.0335x; 1.0335x over previous
"""Trainium2 Bass kernel for batched tanh-query attention.

Per-batch computation (B=8, one batch per NeuronCore, pure data parallel):
    q = tanh(out_state)            [Q, H]    Q=K=2048, H=128
    S = q @ history.T              [Q, K]
    P = softmax(S, axis=K)
    attn = P @ history             [Q, H]

Flash-style in the transposed orientation S_T[k, q] so MM2 needs no
transpose of P.  The kernel is ACT(exp)-rate-limited; everything else is
scheduled around a steady one-exp-per-pair cadence:

  slot (q,p):  MM1 pair p (2x512-col), exp[128,1024] f32->bf16 (ACT),
               DVE lvl1 pair-add + GpSimd lvl2 quad-add for the softmax
               denominator, MM2 accumulation for pair p-2 (lag 2), d
               quad-matmuls at p=4..6 (+1 in next quarter), previous
               quarter's epilogue spread over p1..p3.

Input DMAs ride two rings in parallel (hn chunks on the Sync ring, os
chunks on the GpSimd ring) so the hn stream feeds ht transposes / MM2 at
pair rate while os lands for tanh.  PSUM: 2x2 (st) + 1 (acc) + 1 (dq)
+ 2 (transpose bounce) = 8 banks.
"""

import os
import sys

os.environ.setdefault("NEURON_RT_RESET_CORES", "1")
for _p in ("/opt/trn_rl_repo", "/opt/trn_rl_repo/concourse"):
    if _p not in sys.path:
        sys.path.insert(0, _p)

import numpy as np

N_CORES = 8
SEQ = 2048
H = 128
P = 128
T = SEQ // P          # 16 seq tiles
NQ = 4                # query quarters
QW = SEQ // NQ        # 512
QTPQ = QW // P        # 4 q-tiles per quarter
NPAIR = T // 2        # 8 kb-pairs per quarter

_CACHE = {}


def _build():
    from concourse import bacc, bass, masks, mybir, tile

    f32 = mybir.dt.float32
    bf16 = mybir.dt.bfloat16
    AF = mybir.ActivationFunctionType

    nc = bacc.Bacc("TRN2", target_bir_lowering=False, debug=False,
                   num_devices=N_CORES)
    os_d = nc.dram_tensor("out_state", (SEQ, H), f32, kind="ExternalInput")
    h_d = nc.dram_tensor("history", (SEQ, H), f32, kind="ExternalInput")
    a_d = nc.dram_tensor("attn", (SEQ, H), f32, kind="ExternalOutput")

    with tile.TileContext(nc) as tc:
        with (
            tc.tile_pool(name="const", bufs=1) as constp,
            tc.tile_pool(name="big", bufs=1) as bigp,
            tc.tile_pool(name="stage", bufs=1) as stagep,
            tc.tile_pool(name="work", bufs=2) as workp,
            tc.tile_pool(name="expool", bufs=4) as expool,
            tc.tile_pool(name="dtree", bufs=4) as dtreep,
            tc.tile_pool(name="stp", bufs=2, space=bass.MemorySpace.PSUM) as stp,
            tc.tile_pool(name="pacc", bufs=1, space=bass.MemorySpace.PSUM) as pacc,
            tc.tile_pool(name="psd", bufs=1, space=bass.MemorySpace.PSUM) as psd,
        ):
            # ---- input DMAs: two rings in parallel, ordered by need ----
            os_f = stagep.tile([P, T, H], f32, tag="osf")
            hn_f = stagep.tile([P, T, H], f32, tag="hnf")
            os_v = os_d[:].rearrange("(t p) h -> p t h", p=P)
            hn_v = h_d[:].rearrange("(t p) h -> p t h", p=P)
            # single fast ring (Sync): alternating 4-tile os/hn chunks
            for j in range(4):
                sl = slice(4 * j, 4 * (j + 1))
                nc.sync.dma_start(os_f[:, sl, :], os_v[:, sl, :])
                nc.sync.dma_start(hn_f[:, sl, :], hn_v[:, sl, :])

            # ---- constants ----
            id_bf = constp.tile([P, P], bf16, tag="idb")
            masks.make_identity(nc, id_bf[:])
            id_f32 = constp.tile([P, P], f32, tag="idf")
            masks.make_identity(nc, id_f32[:])
            ones_bf = constp.tile([P, P], bf16, tag="ones")
            nc.vector.memset(ones_bf[:], 1.0)

            # persistent bf16 operands
            hn = bigp.tile([P, T, P], bf16, tag="hn")    # [k_in, t, h] natural
            ht = bigp.tile([P, T, P], bf16, tag="ht")    # [h, t, k_in]
            qT = bigp.tile([P, T, P], bf16, tag="qT")    # [h, t, q_in]
            q_nat = bigp.tile([P, T, H], bf16, tag="qnat")

            # prologue preprocessing: tanh + hn cast per 4-tile chunk
            nc.scalar.activation(q_nat[:, 0:4, :], os_f[:, 0:4, :], AF.Tanh)
            nc.vector.tensor_copy(hn[:, 0:4, :], hn_f[:, 0:4, :])
            nc.scalar.activation(q_nat[:, 4:8, :], os_f[:, 4:8, :], AF.Tanh)
            nc.vector.tensor_copy(hn[:, 4:8, :], hn_f[:, 4:8, :])

            # PE-transpose one [128,128] bf16 tile into transposed layout
            def ptranspose(dst, src):
                tp = psd.tile([P, P], bf16, tag="tp", name="tp", bufs=2)
                nc.tensor.transpose(tp[:], src, id_bf[:])
                nc.vector.tensor_copy(dst, tp[:])

            # upfront: tiles the first A-phase pair needs
            for t in range(QTPQ):
                ptranspose(qT[:, t, :], q_nat[:, t, :])
            ptranspose(ht[:, 0, :], hn[:, 0, :])
            ptranspose(ht[:, 1, :], hn[:, 1, :])

            def late_prep():
                for j in (2, 3):
                    sl = slice(4 * j, 4 * (j + 1))
                    nc.scalar.activation(q_nat[:, sl, :], os_f[:, sl, :],
                                         AF.Tanh)
                    nc.vector.tensor_copy(hn[:, sl, :], hn_f[:, sl, :])

            # aux transpose queue
            aux = []

            def tp_job(kind, t):
                def job():
                    src = hn if kind == "h" else q_nat
                    dst = ht if kind == "h" else qT
                    ptranspose(dst[:, t, :], src[:, t, :])
                return job

            aux.extend(tp_job("h", t) for t in range(2, T))
            aux.extend(tp_job("q", t) for t in range(QTPQ, T))

            def drain_aux(n):
                for _ in range(n):
                    if aux:
                        aux.pop(0)()

            # ---- per-quarter state ----
            ex_tiles = [[None] * NPAIR for _ in range(NQ)]
            lvl2s = [[None] * 4 for _ in range(NQ)]
            l1prev = [None] * NQ
            accs = [None] * NQ
            dqs = [None] * NQ
            aTs_t = [None] * NQ
            dsb_t = [None] * NQ

            def emit_pair(q, p):
                kb0 = 2 * p
                st = stp.tile([P, 2 * QW], f32, tag="st", name="st")
                rhs = qT[:, QTPQ * q: QTPQ * (q + 1), :]
                nc.tensor.matmul(st[:, 0:QW], ht[:, kb0, :], rhs,
                                 start=True, stop=True)
                nc.tensor.matmul(st[:, QW:], ht[:, kb0 + 1, :], rhs,
                                 start=True, stop=True)
                ex = expool.tile([P, 2 * QW], bf16, tag="ex", name="ex")
                nc.scalar.activation(ex[:], st[:], AF.Exp)
                ex_tiles[q][p] = ex
                # d partial sums: DVE pair add, GpSimd quad add
                t1 = dtreep.tile([P, QW], bf16, tag="l1", name="t1", bufs=2)
                nc.vector.tensor_add(t1[:], ex[:, 0:QW], ex[:, QW:])
                if l1prev[q] is None:
                    l1prev[q] = t1
                else:
                    t2 = dtreep.tile([P, QW], bf16, tag="l2", name="t2",
                                     bufs=4)
                    eng = nc.vector if q == NQ - 1 else nc.gpsimd
                    eng.tensor_add(t2[:], l1prev[q][:], t1[:])
                    l1prev[q] = None
                    lvl2s[q][p // 2] = t2

            def emit_mm2(q, kb):
                if accs[q] is None:
                    accs[q] = pacc.tile([P, QW], f32, tag="acc",
                                        name=f"acc{q}")
                ex = ex_tiles[q][kb // 2]
                nc.tensor.matmul(
                    accs[q][:], hn[:, kb, :],
                    ex[:, QW * (kb % 2): QW * (kb % 2 + 1)],
                    start=(kb == 0), stop=(kb == T - 1))

            def emit_dmm(q, j):
                if dqs[q] is None:
                    dqs[q] = psd.tile([P, QW], f32, tag="dq", name=f"dq{q}")
                nc.tensor.matmul(dqs[q][:], ones_bf[:], lvl2s[q][j][:],
                                 start=(j == 0), stop=(j == 3))

            def emit_epi_head(q):
                # move acc + d row out of PSUM, freeing acc/dq slots
                d_sb = workp.tile([1, QW], f32, tag="dsb", name=f"dsb{q}")
                nc.vector.tensor_copy(d_sb[:], dqs[q][0:1, :])
                dsb_t[q] = d_sb
                aTs = workp.tile([P, QW], bf16, tag="aTs", name=f"aTs{q}")
                nc.vector.tensor_copy(aTs[:], accs[q][:])
                aTs_t[q] = aTs

            def emit_epi_tile(q, t):
                dps = psd.tile([P, 1], f32, tag="tp", name="dps", bufs=2)
                nc.tensor.transpose(dps[:], dsb_t[q][0:1, P * t: P * (t + 1)],
                                    id_f32[0:1, 0:1])
                rc = workp.tile([P, 1], f32, tag="rc", name="rc", bufs=4)
                nc.vector.reciprocal(rc[:], dps[:])
                tp = psd.tile([P, P], bf16, tag="tp", name="etp", bufs=2)
                nc.tensor.transpose(tp[:], aTs_t[q][:, P * t: P * (t + 1)],
                                    id_bf[:])
                ot = workp.tile([P, P], f32, tag="ot", name="ot", bufs=4)
                nc.vector.tensor_scalar_mul(ot[:], tp[:], rc[:])
                row0 = q * QW + P * t
                nc.sync.dma_start(a_d[row0: row0 + P, :], ot[:])

            # ---- emission schedule ----
            for q in range(NQ):
                for p in range(NPAIR):
                    if q == 0 and p >= 1:
                        drain_aux(2)          # ht transposes, 1 pair ahead
                    emit_pair(q, p)
                    if p >= 2:
                        emit_mm2(q, 2 * p - 4)
                        emit_mm2(q, 2 * p - 3)
                    if p == 0 and q >= 1:
                        emit_mm2(q - 1, 12)
                        emit_mm2(q - 1, 13)
                        emit_dmm(q - 1, 0)
                        emit_dmm(q - 1, 1)
                    if p == 1 and q >= 1:
                        emit_mm2(q - 1, 14)
                        emit_mm2(q - 1, 15)
                        emit_dmm(q - 1, 2)
                        emit_dmm(q - 1, 3)
                        emit_epi_head(q - 1)
                    if q >= 1 and p in (2, 3):
                        emit_epi_tile(q - 1, 2 * (p - 2))
                        emit_epi_tile(q - 1, 2 * (p - 2) + 1)
                    if q == NQ - 1 and p in (5, 6, 7):
                        emit_dmm(q, p - 5)
                    if q == 0 and p == 2:
                        late_prep()
                    if q == 0 and p == 7:
                        drain_aux(4)          # qT tiles 4-7 before A(1)
                    if q >= 1 and p in (3, 4, 5, 6):
                        drain_aux(1)          # remaining qT transposes

            # ---- tail: finish quarter 3 ----
            for kb in (12, 13, 14, 15):
                emit_mm2(3, kb)
            emit_dmm(3, 3)
            emit_epi_head(3)
            for t in range(QTPQ):
                emit_epi_tile(3, t)
            while aux:
                aux.pop(0)()

    nc.compile()
    return nc


def _get_nc():
    if "nc" not in _CACHE:
        _CACHE["nc"] = _build()
    return _CACHE["nc"]


def _run(out_state, history, trace=False):
    from concourse.bass_utils import run_bass_kernel_spmd

    nc = _get_nc()
    out_state = np.ascontiguousarray(out_state, dtype=np.float32)
    history = np.ascontiguousarray(history, dtype=np.float32)
    in_maps = [
        {"out_state": out_state[b], "history": history[b]}
        for b in range(N_CORES)
    ]
    res = run_bass_kernel_spmd(nc, in_maps, core_ids=list(range(N_CORES)),
                               trace=trace)
    attn = np.stack([res.results[b]["attn"] for b in range(N_CORES)], axis=0)
    return attn.astype(np.float32), res


def kernel(out_state, history):
    try:
        attn, _ = _run(out_state, history)
    except Exception:
        # one retry, e.g. if a previous process left a core wedged
        attn, _ = _run(out_state, history)
    return attn


# revision 13
# speedup vs baseline: 1.0576x; 1.0234x over previous
"""Trainium2 Bass kernel for batched tanh-query attention.

Per-batch computation (B=8, one batch per NeuronCore, pure data parallel):
    q = tanh(out_state)            [Q, H]    Q=K=2048, H=128
    S = q @ history.T              [Q, K]
    P = softmax(S, axis=K)
    attn = P @ history             [Q, H]

Flash-style, no HBM intermediates, computed in the transposed orientation
S_T[k, q] so the second matmul needs no transpose of P. Queries are
processed in 4 quarters of 512 columns; each quarter runs two software
phases that overlap across quarters:
  A(q): per kb-pair  S_T = ht[kb].T @ qT  (PE) -> exp FD=1024 (ACT, bf16)
        + two levels of bf16 pair-adds on DVE for the softmax denominator
  B(q): 16 accumulating  attn_T += hn[kb].T @ expS  matmuls + 4 ones-matmuls
        for d, emitted in two dense batches inside A(q+1) so PE runs them
        back-to-back while ACT keeps computing exps.
Epilogue per quarter (PE-transpose attn_T / d back to q-major, 1/d scale,
DMA out) is drained into later quarters' A phases.
"""

import os
import sys

os.environ.setdefault("NEURON_RT_RESET_CORES", "1")
for _p in ("/opt/trn_rl_repo", "/opt/trn_rl_repo/concourse"):
    if _p not in sys.path:
        sys.path.insert(0, _p)

import numpy as np

N_CORES = 8
SEQ = 2048
H = 128
P = 128
T = SEQ // P          # 16 seq tiles
NQ = 4                # query quarters
QW = SEQ // NQ        # 512
QTPQ = QW // P        # 4 q-tiles per quarter
NPAIR = T // 2        # 8 kb-pairs per quarter

_CACHE = {}


def _build():
    from concourse import bacc, bass, masks, mybir, tile

    f32 = mybir.dt.float32
    bf16 = mybir.dt.bfloat16
    AF = mybir.ActivationFunctionType

    nc = bacc.Bacc("TRN2", target_bir_lowering=False, debug=False,
                   num_devices=N_CORES)
    os_d = nc.dram_tensor("out_state", (SEQ, H), f32, kind="ExternalInput")
    h_d = nc.dram_tensor("history", (SEQ, H), f32, kind="ExternalInput")
    a_d = nc.dram_tensor("attn", (SEQ, H), f32, kind="ExternalOutput")

    with tile.TileContext(nc) as tc:
        with (
            tc.tile_pool(name="const", bufs=1) as constp,
            tc.tile_pool(name="big", bufs=1) as bigp,
            tc.tile_pool(name="stage", bufs=2) as stagep,
            tc.tile_pool(name="work", bufs=6) as workp,
            tc.tile_pool(name="expool", bufs=13) as expool,
            tc.tile_pool(name="dtree", bufs=8) as dtreep,
            tc.tile_pool(name="ps", bufs=2, space=bass.MemorySpace.PSUM) as psp,
            tc.tile_pool(name="psacc", bufs=2, space=bass.MemorySpace.PSUM) as pacc,
            tc.tile_pool(name="psd", bufs=2, space=bass.MemorySpace.PSUM) as psd,
        ):
            id_f32 = constp.tile([P, P], f32, tag="idf")
            masks.make_identity(nc, id_f32[:])
            id_bf = constp.tile([P, P], bf16, tag="idb")
            masks.make_identity(nc, id_bf[:])
            ones_bf = constp.tile([P, P], bf16, tag="ones")
            nc.vector.memset(ones_bf[:], 1.0)

            # persistent bf16 operands
            hn = bigp.tile([P, T, P], bf16, tag="hn")    # [k_in, t, h] natural
            ht = bigp.tile([P, T, P], bf16, tag="ht")    # [h, t, k_in]
            qT = bigp.tile([P, T, P], bf16, tag="qT")    # [h, t, q_in]

            # ---- load + preprocess (chunked so compute starts early) ----
            os_f = stagep.tile([P, T, H], f32, tag="ldin")
            hn_f = stagep.tile([P, T, H], f32, tag="ldin")
            os_v = os_d[:].rearrange("(t p) h -> p t h", p=P)
            hn_v = h_d[:].rearrange("(t p) h -> p t h", p=P)
            for j in range(4):
                sl = slice(4 * j, 4 * (j + 1))
                nc.sync.dma_start(os_f[:, sl, :], os_v[:, sl, :])
                nc.sync.dma_start(hn_f[:, sl, :], hn_v[:, sl, :])

            q_nat = stagep.tile([P, T, H], bf16, tag="qnat")
            for j in range(2):
                sl = slice(4 * j, 4 * (j + 1))
                nc.scalar.activation(q_nat[:, sl, :], os_f[:, sl, :], AF.Tanh)
                nc.vector.tensor_copy(hn[:, sl, :], hn_f[:, sl, :])

            def late_prep():
                for j in range(2, 4):
                    sl = slice(4 * j, 4 * (j + 1))
                    nc.scalar.activation(q_nat[:, sl, :], os_f[:, sl, :],
                                         AF.Tanh)
                    nc.vector.tensor_copy(hn[:, sl, :], hn_f[:, sl, :])

            # PE-transpose one [128,128] bf16 tile into a transposed layout
            def ptranspose(dst, src):
                tp = psd.tile([P, P], bf16, tag="dbc", name="tp")
                nc.tensor.transpose(tp[:], src, id_bf[:])
                nc.vector.tensor_copy(dst, tp[:])

            # aux work queue: input transposes now, epilogue tiles later
            aux = []

            def drain_aux(n):
                for _ in range(n):
                    if aux:
                        aux.pop(0)()

            def tp_job(kind, t):
                def job():
                    src = hn if kind == "h" else q_nat
                    dst = ht if kind == "h" else qT
                    ptranspose(dst[:, t, :], src[:, t, :])
                return job

            # upfront: tiles the first A-phase pairs need
            for t in range(QTPQ):
                ptranspose(qT[:, t, :], q_nat[:, t, :])
            for t in range(2):
                ptranspose(ht[:, t, :], hn[:, t, :])
            aux.extend(tp_job("h", t) for t in range(2, T))
            aux.extend(tp_job("q", t) for t in range(QTPQ, T))

            # ---- epilogue helper: one output q-tile of 128 rows ----
            def emit_epi(q, t, aT_sb, d_sb):
                dps = pacc.tile([P, 1], f32, tag="acc", name="dps")
                nc.tensor.transpose(dps[:], d_sb[0:1, P * t: P * (t + 1)],
                                    id_f32[0:1, 0:1])
                rc = workp.tile([P, 1], f32, tag="rc", name="rc")
                nc.vector.reciprocal(rc[:], dps[:])
                aps = pacc.tile([P, P], f32, tag="acc", name="aps")
                nc.tensor.transpose(aps[:], aT_sb[:, P * t: P * (t + 1)],
                                    id_f32[:])
                ot = workp.tile([P, P], f32, tag="ot", name="ot")
                nc.vector.tensor_scalar_mul(ot[:], aps[:], rc[:])
                row0 = q * QW + P * t
                nc.sync.dma_start(a_d[row0: row0 + P, :], ot[:])

            # ---- build per-quarter phase closures ----
            ex_tiles = [[] for _ in range(NQ)]
            lvl2s = [[] for _ in range(NQ)]
            accs = [None] * NQ
            dqs = [None] * NQ
            l1prev = [None] * NQ

            def emit_pair(q, p):
                kb0 = 2 * p
                st = psp.tile([P, 2 * QW], f32, tag="st", name="st")
                rhs = qT[:, QTPQ * q: QTPQ * (q + 1), :]
                nc.tensor.matmul(st[:, 0:QW], ht[:, kb0, :], rhs,
                                 start=True, stop=True)
                nc.tensor.matmul(st[:, QW:], ht[:, kb0 + 1, :], rhs,
                                 start=True, stop=True)
                ex = expool.tile([P, 2 * QW], bf16, tag="ex", name="ex")
                nc.scalar.activation(ex[:], st[:], AF.Exp)
                ex_tiles[q].append(ex)
                # d: in-tile pair add, then quad add (DVE, bf16)
                t1 = dtreep.tile([P, QW], bf16, tag="l1", name="t1")
                nc.vector.tensor_add(t1[:], ex[:, 0:QW], ex[:, QW:])
                if l1prev[q] is None:
                    l1prev[q] = t1
                else:
                    t2 = dtreep.tile([P, QW], bf16, tag="l2", name="t2")
                    nc.vector.tensor_add(t2[:], l1prev[q][:], t1[:])
                    l1prev[q] = None
                    lvl2s[q].append(t2)

            def emit_B(q, i):
                # accumulators are allocated at first write: their lifetime
                # (B(q) start .. epilogue copy) never overlaps the next
                # quarter's, so one PSUM slot per tag suffices
                if accs[q] is None:
                    accs[q] = pacc.tile([P, QW], f32, tag="acc",
                                        name=f"acc{q}")
                    dqs[q] = psd.tile([P, QW], f32, tag="dbc", name=f"dq{q}")
                # batch i: 8 accumulating MM2s + two d quad matmuls
                for kb in range(8 * i, 8 * (i + 1)):
                    nc.tensor.matmul(
                        accs[q][:], hn[:, kb, :],
                        ex_tiles[q][kb // 2][:, QW * (kb % 2): QW * (kb % 2 + 1)],
                        start=(kb == 0), stop=(kb == T - 1))
                for j in (2 * i, 2 * i + 1):
                    nc.tensor.matmul(dqs[q][:], ones_bf[:], lvl2s[q][j][:],
                                     start=(j == 0), stop=(j == 3))
                if i == 1:
                    # move accumulators to SBUF, queue epilogue tiles
                    aT_sb = workp.tile([P, QW], f32, tag="atsb",
                                       name=f"aT{q}")
                    nc.vector.tensor_copy(aT_sb[:], accs[q][:])
                    d_sb = workp.tile([P, QW], f32, tag="dsb", name=f"d{q}")
                    nc.vector.tensor_copy(d_sb[:], dqs[q][:])
                    aux.extend(
                        (lambda t=t, a=aT_sb, d=d_sb, q=q: emit_epi(q, t, a, d))
                        for t in range(QTPQ))

            # ---- emission schedule ----
            for q in range(NQ):
                for p in range(NPAIR):
                    if q == 0 and p >= 1:
                        # ht transposes, one pair ahead of their consumer
                        drain_aux(2)
                    emit_pair(q, p)
                    if q == 0 and p == 2:
                        late_prep()
                    if q > 0 and p in (1, 3):
                        emit_B(q - 1, p // 2)
                    if q == NQ - 1 and p == 5:
                        emit_B(q, 0)
                    if p >= 4:
                        drain_aux(2)
            emit_B(NQ - 1, 1)
            while aux:
                aux.pop(0)()

    nc.compile()
    return nc


def _get_nc():
    if "nc" not in _CACHE:
        _CACHE["nc"] = _build()
    return _CACHE["nc"]


def _run(out_state, history, trace=False):
    from concourse.bass_utils import run_bass_kernel_spmd

    nc = _get_nc()
    out_state = np.ascontiguousarray(out_state, dtype=np.float32)
    history = np.ascontiguousarray(history, dtype=np.float32)
    in_maps = [
        {"out_state": out_state[b], "history": history[b]}
        for b in range(N_CORES)
    ]
    res = run_bass_kernel_spmd(nc, in_maps, core_ids=list(range(N_CORES)),
                               trace=trace)
    attn = np.stack([res.results[b]["attn"] for b in range(N_CORES)], axis=0)
    return attn.astype(np.float32), res


def kernel(out_state, history):
    try:
        attn, _ = _run(out_state, history)
    except Exception:
        # one retry, e.g. if a previous process left a core wedged
        attn, _ = _run(out_state, history)
    return attn

